# revision 11
# baseline (speedup 1.0000x reference)
"""Neural ODE Euler integration on 8 Trainium2 NeuronCores.

h_{n+1} = h_n + 0.1 * tanh(h_n @ W[k].T + b[k]),  k = n // 10,  100 steps.
x: [1024, 1024], W: [10, 1024, 1024], b: [10, 1024].
Returns (features [1024,1024], traj [101,1024,1024]) like the reference.

Strategy: data-parallel over batch (128 rows per core), weights replicated.
On-chip state is kept transposed (hT[i, b], i on partitions in 8 chunks of
128) so the per-step matmul uses hT chunks as the stationary operand and a
host-pre-transposed W as the moving operand with free dim 512, hitting the
float32r 1-cycle/row tensor-engine path. The carried state stays full fp32;
a per-chunk fp32r rounding copy feeds the matmuls. tanh+bias runs on the
scalar engine in transposed layout (bias is per-partition there) and the
Euler update is a fused (t*dt)+h op on the vector engine. All tail work is
per-128-chunk so the next step's matmuls start as soon as chunk 0 is ready.
"""

import contextlib

import numpy as np

import concourse.bacc as bacc
import concourse.mybir as mybir
import concourse.tile as tile
from concourse import bass_utils

F32 = mybir.dt.float32
F32R = mybir.dt.float32r

N_STEPS = 100
NUM_VALS = 10
STEPS_PER_SLAB = N_STEPS // NUM_VALS
DT = 0.1
B, D = 1024, 1024
N_CORES = 8
BL = B // N_CORES  # 128 batch rows per core
NJ = D // 128  # 8 partition chunks of the D axis
HALF = 512  # matmul moving free dim (PSUM bank limit for f32)

_cache: dict = {}


def _build_nc(reps: int = 1):
    """reps>1 wraps the 100-step pass in a hardware loop (timing use only;
    the state is re-initialized from DRAM each pass)."""
    nc = bacc.Bacc("TRN2", target_bir_lowering=False, debug=False)
    xT_d = nc.dram_tensor("xT", [128, D], F32, kind="ExternalInput").ap()
    wT_d = nc.dram_tensor("WT", [NUM_VALS, D, D], F32R, kind="ExternalInput").ap()
    bias_d = nc.dram_tensor("bias", [128, NUM_VALS * NJ], F32, kind="ExternalInput").ap()
    ident_d = nc.dram_tensor("ident", [128, 128], F32, kind="ExternalInput").ap()
    traj_d = nc.dram_tensor(
        "trajT", [N_STEPS + 1, D, BL], F32, kind="ExternalOutput"
    ).ap()

    with tile.TileContext(nc) as tc:
        with (
            tc.tile_pool(name="const", bufs=1) as constp,
            tc.tile_pool(name="state", bufs=2) as statep,
            tc.tile_pool(name="wslab", bufs=2) as wp,
            tc.tile_pool(name="ysb", bufs=2) as ysbp,
            tc.tile_pool(name="tt", bufs=2) as ttp,
            tc.tile_pool(name="ypsum", bufs=4, space="PSUM") as yp,
            tc.tile_pool(name="tpsum", bufs=4, space="PSUM") as tpp,
        ):
            bias_sb = constp.tile([128, NUM_VALS * NJ], F32)
            nc.sync.dma_start(bias_sb[:], bias_d[:])
            ident_sb = constp.tile([128, 128], F32)
            nc.sync.dma_start(ident_sb[:], ident_d[:])

            def load_slab(k):
                t = wp.tile([128, NJ * D], F32R, tag="wslab", name=f"w{k}")
                nc.sync.dma_start(t[:], wT_d[k].rearrange("(i p) o -> p i o", p=128))
                return t

            def load_state():
                uT0 = statep.tile([128, D], F32, tag="state", name="uT_init")
                nc.sync.dma_start(uT0[:], xT_d[:])
                uTr0 = statep.tile([128, D], F32R, tag="stater", name="uTr_init")
                nc.gpsimd.tensor_copy(uTr0[:], uT0[:])
                return uT0, uTr0

            slabs: list = [None] * NUM_VALS
            uT = uTr = None
            if reps == 1:
                slabs[0] = load_slab(0)
                uT, uTr = load_state()
                # traj[0] = x
                nc.sync.dma_start(
                    traj_d[0].rearrange("(j p) b -> p j b", p=128),
                    uT[:].rearrange("p (j b) -> p j b", j=NJ),
                )

            loop_cm = tc.For_i(0, reps, 1) if reps > 1 else contextlib.nullcontext()
            with loop_cm:
                for n in range(N_STEPS):
                    k = n // STEPS_PER_SLAB
                    if reps > 1 and n == 0:
                        slabs[0] = load_slab(0)
                        uT, uTr = load_state()
                    if n % STEPS_PER_SLAB == 0 and k + 1 < NUM_VALS:
                        slabs[k + 1] = load_slab(k + 1)
                    w = slabs[k]

                    # y[b, o] = sum_i u[i, b] * WT[i, o], accumulated over the
                    # 8 i-chunks. Half h runs fully before half h+1 so its
                    # PSUM->SBUF copy and chunk tails overlap the other
                    # half's matmuls.
                    yps = [
                        yp.tile([128, HALF], F32, tag="y", name=f"y{n}_{h}")
                        for h in range(2)
                    ]
                    for h in range(2):
                        for ic in range(NJ):
                            nc.tensor.matmul(
                                yps[h][:],
                                uTr[:, ic * 128 : (ic + 1) * 128],
                                w[:, ic * D + h * HALF : ic * D + (h + 1) * HALF],
                                start=(ic == 0),
                                stop=(ic == NJ - 1),
                            )

                    # quarter-granularity PSUM->SBUF copies for finer overlap,
                    # alternating DVE/ACT to keep DVE off the critical path
                    ysb = ysbp.tile([128, D], F32, tag="ysb")
                    for q in range(4):
                        src_sl = yps[q // 2][:, (q % 2) * 256 : (q % 2 + 1) * 256]
                        dst_sl = ysb[:, q * 256 : (q + 1) * 256]
                        if q % 2 == 0:
                            nc.vector.tensor_copy(dst_sl, src_sl)
                        else:
                            nc.scalar.copy(dst_sl, src_sl)

                    # per chunk: PE transpose (PSUM) -> tanh+bias to SBUF
                    # (ACT; SBUF dest avoids PSUM bank serialization between
                    # the in-flight chunks) -> Euler update computed twice in
                    # parallel: DVE writes the fp32r copy that feeds the next
                    # step's matmuls, GPSIMD writes the fp32 carry. Next
                    # step's matmul on chunk ic depends only on chunk ic's
                    # tanh+DVE update, so the PE pipeline never drains.
                    uT_new = statep.tile([128, D], F32, tag="state")
                    uTr_new = statep.tile([128, D], F32R, tag="stater")
                    tt = ttp.tile([128, D], F32, tag="tt")
                    tps = [
                        tpp.tile([128, HALF], F32, tag="t", name=f"t{n}_{h}")
                        for h in range(2)
                    ]
                    for j in range(NJ):
                        nc.tensor.transpose(
                            tps[j // 4][:, (j % 4) * 128 : (j % 4 + 1) * 128],
                            ysb[:, j * 128 : (j + 1) * 128],
                            ident_sb[:],
                        )
                    for j in range(NJ):
                        t_sl = tps[j // 4][:, (j % 4) * 128 : (j % 4 + 1) * 128]
                        sl = slice(j * 128, (j + 1) * 128)
                        col = k * NJ + j
                        nc.scalar.activation(
                            tt[:, sl],
                            t_sl,
                            mybir.ActivationFunctionType.Tanh,
                            bias=bias_sb[:, col : col + 1],
                            scale=1.0,
                        )
                        nc.vector.tensor_add(uTr_new[:, sl], tt[:, sl], uT[:, sl])
                        nc.gpsimd.tensor_add(uT_new[:, sl], tt[:, sl], uT[:, sl])

                    nc.sync.dma_start(
                        traj_d[n + 1].rearrange("(j p) b -> p j b", p=128),
                        uT_new[:].rearrange("p (j b) -> p j b", j=NJ),
                    )
                    uT = uT_new
                    uTr = uTr_new

    nc.compile()
    return nc


def kernel(x: np.ndarray, W: np.ndarray, b: np.ndarray):
    x = np.asarray(x, dtype=np.float32)
    W = np.asarray(W, dtype=np.float32)
    b = np.asarray(b, dtype=np.float32)

    if "nc" not in _cache:
        _cache["nc"] = _build_nc()
    nc = _cache["nc"]

    # u = h/DT substitution: u_{n+1} = u_n + tanh((DT*W) u_n + b).
    # WT[k, i, o] = DT * W[k, o, i]
    WT = np.ascontiguousarray(DT * W.transpose(0, 2, 1))
    # bias_re[p, k*8+j] = b[k, j*128+p]
    bias_re = np.ascontiguousarray(
        b.reshape(NUM_VALS, NJ, 128).transpose(2, 0, 1).reshape(128, NUM_VALS * NJ)
    )
    ident = np.eye(128, dtype=np.float32)

    in_maps = []
    for c in range(N_CORES):
        xs = x[c * BL : (c + 1) * BL]  # [128(b), 1024(i)]
        # xT[p, j*128 + b_l] = x_shard[b_l, j*128+p]
        xT = np.ascontiguousarray(
            (1.0 / DT) * xs.reshape(BL, NJ, 128).transpose(2, 1, 0).reshape(128, D)
        )
        in_maps.append({"xT": xT, "WT": WT, "bias": bias_re, "ident": ident})

    res = bass_utils.run_bass_kernel_spmd(nc, in_maps, core_ids=list(range(N_CORES)))
    _cache["last_results"] = res

    traj = np.empty((N_STEPS + 1, B, D), dtype=np.float32)
    for c in range(N_CORES):
        tT = res.results[c]["trajT"]  # [101, 1024(i), 128(b_l)], in u = h/DT scale
        traj[:, c * BL : (c + 1) * BL, :] = DT * tT.transpose(0, 2, 1)
    features = traj[N_STEPS].copy()
    return features, traj
